# revision 1
# baseline (speedup 1.0000x reference)
"""Trainium2 Bass kernel for nn_Detect_50431505989817 (YOLO-style detect head).

Computes, for each of 8 images (one per NeuronCore, batch-parallel):
  level0: 1x1 conv (W0 [1548,256]) over x0 [256,64,64] + decode -> [73728, 86]
  level1: 1x1 conv (W1 [1548,512]) over x1 [512,32,32] + decode -> [18432, 86]
  concat -> out [92160, 86]; host stacks cores -> [8, 92160, 86].

Design notes:
  - matmul: stationary = x tile [K=c, 128 hw] in fp16 (same 11-bit mantissa
    as TF32 -> identical rounding error on this data, half the HBM bytes,
    full PE rate), moving = W^T chunk [K=c, n_anchors*86] fp16.
    hw is interleaved: partition p
    covers hw = 512*blk + 4*p + j, with j in [0,4) living in the free dim
    (PSUM bank j).  That makes each partition of the decoded stage tile hold
    4 consecutive output rows => 1376B contiguous DMA runs (full HBM BW;
    <512B runs pay 2x).
  - decode: one ACT Sigmoid per (block, o-chunk) covers xy/conf/cls; wh uses
    exp(t) = sig/(1-sig) on DVE (avoids the 1283ns ACT table swap between the
    Sigmoid and Exp LUT tables); xy adds a host-precomputed grid via fused
    scalar_tensor_tensor; angle is a DVE add reading raw PSUM.
  - host folds anchors/strides/grid into small constant inputs; a nonzero
    conv bias is handled exactly via an appended ones-row/bias-row (K+1).
"""

import math

import numpy as np

import concourse.mybir as mybir
import concourse.tile as tile
from concourse import bacc, bass_utils

F32 = mybir.dt.float32
F16 = mybir.dt.float16
AFT = mybir.ActivationFunctionType
ALU = mybir.AluOpType

NCLS = 80
NA = 18
NCH = 86  # 5 + 1 + NCLS
STRIDES = [8.0, 16.0]
SXY = [1.2, 1.1]
ANCH = [[[10.0, 13.0], [16.0, 30.0], [33.0, 23.0]],
        [[30.0, 61.0], [62.0, 45.0], [59.0, 119.0]]]
ANGLES = [math.pi / 180.0 * a for a in (-60.0, -30.0, 0.0, 30.0, 60.0, 90.0)]

LEVELS = [
    dict(C=256, G=64, HW=4096, s=STRIDES[0], sxy=SXY[0], row0=0),
    dict(C=512, G=32, HW=1024, s=STRIDES[1], sxy=SXY[1], row0=NA * 4096),
]
OUT_ROWS = NA * (4096 + 1024)  # 92160

# o-chunks: (first anchor, n anchors)
OCH = [(0, 5), (5, 5), (10, 5), (15, 3)]

_PROG_CACHE = {}


def _build_program(use_bias: bool):
    nc = bacc.Bacc("TRN2", target_bir_lowering=False, debug=False)

    xs_d, wt_d = [], []
    for li, lv in enumerate(LEVELS):
        K = lv["C"] + (1 if use_bias else 0)
        xs_d.append(nc.dram_tensor(f"xs{li}", [K, lv["HW"]], F16, kind="ExternalInput"))
        wt_d.append(nc.dram_tensor(f"wt{li}", [K, NA * NCH], F16, kind="ExternalInput"))
    # all decode constants packed into one tensor: one DMA, >=512B rows
    # layout: [grid0(64) | grid1(16) | cwh0(36) | cwh1(36) | cang0(18) | cang1(18)]
    cst_d = nc.dram_tensor("cst", [128, 188], F32, kind="ExternalInput")
    out_d = nc.dram_tensor("out", [OUT_ROWS, NCH], F32, kind="ExternalOutput")

    with tile.TileContext(nc) as tc:
        with (
            tc.tile_pool(name="const", bufs=1) as cpool,
            tc.tile_pool(name="stage", bufs=8) as spool,
            tc.tile_pool(name="tmp", bufs=6) as tpool,
            tc.tile_pool(name="psum", bufs=2, space="PSUM") as ppool,
        ):
            zb = cpool.tile([128, 1], F32, tag="zb")
            nc.gpsimd.memset(zb[:], 0.0)

            # resident inputs: packed decode constants first (one small DMA)
            # so the first tile's decode isn't gated on the multi-MB x/W loads
            cst = cpool.tile([128, 188], F32, tag="cst")
            nc.sync.dma_start(cst[:], cst_d.ap()[:])
            grid_t = [cst[:, 0:64], cst[:, 64:80]]
            cwh_t = [cst[:, 80:116], cst[:, 116:152]]
            cang_t = [cst[:, 152:170], cst[:, 170:188]]

            xs_t, wt_t = [], []
            for li, lv in enumerate(LEVELS):
                K = lv["C"] + (1 if use_bias else 0)
                kch = [(k, min(128, K - k)) for k in range(0, K, 128)]
                xts, wts = [], []
                for k0, kc in kch:
                    # fp16 tiles: same 11-bit mantissa as f32r/TF32 (verified
                    # identical decode error on this data) at half the HBM
                    # bytes, full-rate on the PE, and FWL-capable weight loads
                    wt = cpool.tile([kc, NA * NCH], F16, tag=f"wt{li}_{k0}")
                    nc.sync.dma_start(wt[:], wt_d[li].ap()[k0:k0 + kc, :])
                    wts.append(wt)
                    xt = cpool.tile([kc, lv["HW"]], F16, tag=f"xs{li}_{k0}")
                    nc.sync.dma_start(xt[:], xs_d[li].ap()[k0:k0 + kc, :])
                    xts.append(xt)
                xs_t.append(xts)
                wt_t.append(wts)

            for li, lv in enumerate(LEVELS):
                HW, s, sxy, row0 = lv["HW"], lv["s"], lv["sxy"], lv["row0"]
                nb = HW // 512
                nk = len(xs_t[li])
                # [K, HW] viewed as [K, hw//4, j]
                xs_r = [xt.rearrange("k (h j) -> k h j", j=4) for xt in xs_t[li]]
                # DRAM rows of this level as [anchor, block, 128, 344]
                dst_l = out_d.ap()[row0:row0 + NA * HW, :].rearrange(
                    "(a b h j) c -> a b h (j c)", a=NA, b=nb, j=4)

                for b in range(nb):
                    for ci, (a0, na) in enumerate(OCH):
                        P = ppool.tile([128, 2048], F32, tag="psum")
                        for j in range(4):
                            for ki in range(nk):
                                nc.tensor.matmul(
                                    P[:, 512 * j: 512 * j + na * NCH],
                                    xs_r[ki][:, 128 * b: 128 * (b + 1), j],
                                    wt_t[li][ki][:, NCH * a0: NCH * (a0 + na)],
                                    start=(ki == 0), stop=(ki == nk - 1),
                                )

                        S = spool.tile([128, na * 4 * NCH], F32, tag="S")
                        # psum viewed [p, j, a, c] and [p, a, j, c]
                        Pj = P.rearrange("p (j q) -> p j q", q=512)[:, :, 0:na * NCH] \
                            .rearrange("p j (a c) -> p j a c", c=NCH)
                        Pa = Pj.rearrange("p j a c -> p a j c")
                        # stage S layout per partition: [a][j][c]
                        Sa = S.rearrange("p (a j c) -> p a j c", j=4, c=NCH)
                        Sj = Sa.rearrange("p a j c -> p j a c")

                        nc.scalar.activation(Sj, Pj, AFT.Sigmoid, bias=zb[:])

                        # xy: sig*(sxy*s) + grid(hw)
                        gb = grid_t[li][:, 8 * b: 8 * b + 8] \
                            .rearrange("p (a j c) -> p a j c", a=1, c=2) \
                            .broadcast_to([128, na, 4, 2])
                        nc.vector.scalar_tensor_tensor(
                            Sa[:, :, :, 0:2], Sa[:, :, :, 0:2], sxy * s, gb,
                            ALU.mult, ALU.add)

                        # wh: exp(t)*w = w * sig/(1-sig)
                        T = tpool.tile([128, na * 8], F32, tag="T")
                        Tr = T.rearrange("p (a j c) -> p a j c", j=4, c=2)
                        cwb = cwh_t[li][:, 2 * a0: 2 * (a0 + na)] \
                            .rearrange("p (a j c) -> p a j c", j=1, c=2) \
                            .broadcast_to([128, na, 4, 2])
                        nc.vector.tensor_scalar(
                            Tr, Sa[:, :, :, 2:4], -1.0, 1.0, ALU.mult, ALU.add)
                        nc.vector.reciprocal_approx_fast(T[:], T[:])
                        nc.vector.tensor_tensor(Tr, Tr, cwb, ALU.mult)
                        nc.vector.tensor_tensor(
                            Sa[:, :, :, 2:4], Sa[:, :, :, 2:4], Tr, ALU.mult)

                        # angle: t + aa (raw PSUM read)
                        cab = cang_t[li][:, a0:a0 + na] \
                            .rearrange("p (a j c) -> p a j c", j=1, c=1) \
                            .broadcast_to([128, na, 4, 1])
                        nc.vector.tensor_tensor(
                            Sa[:, :, :, 4:5], Pa[:, :, :, 4:5], cab, ALU.add)

                        # store: [p, a, j*c] -> rows (a0+i)*HW + 512b + 4p + j
                        # (partition dim must stay outermost on the SBUF side)
                        dst = dst_l[a0:a0 + na, b, :, :].rearrange("a h q -> h a q")
                        src = S.rearrange("p (a q) -> p a q", q=4 * NCH)
                        nc.sync.dma_start(dst, src)

    nc.compile()
    return nc


def _get_program(use_bias: bool):
    key = bool(use_bias)
    if key not in _PROG_CACHE:
        _PROG_CACHE[key] = _build_program(key)
    return _PROG_CACHE[key]


def _host_consts():
    """Shared (per-core-identical) packed constant input (see cst layout)."""
    grids, cwhs, cangs = [], [], []
    for li, lv in enumerate(LEVELS):
        G, HW, s, sxy = lv["G"], lv["HW"], lv["s"], lv["sxy"]
        nb = HW // 512
        # grid[p, 8b + 2j + c] = value_c(hw = 512b + 4p + j)
        p = np.arange(128)
        b = np.arange(nb)
        j = np.arange(4)
        hw = 512 * b[None, :, None] + 4 * p[:, None, None] + j[None, None, :]
        gx = (hw % G - (sxy - 1.0) / 2.0) * s
        gy = (hw // G - (sxy - 1.0) / 2.0) * s
        grid = np.stack([gx, gy], axis=-1)  # [128, nb, 4, 2]
        grids.append(grid.reshape(128, 8 * nb).astype(np.float32))

        wh = np.array([ANCH[li][a // 6] for a in range(NA)], dtype=np.float32)
        cwhs.append(np.broadcast_to(wh.reshape(1, 2 * NA), (128, 2 * NA)))
        ang = np.array([ANGLES[a % 6] for a in range(NA)], dtype=np.float32)
        cangs.append(np.broadcast_to(ang.reshape(1, NA), (128, NA)))
    cst = np.concatenate(grids + cwhs + cangs, axis=1).astype(np.float32)
    return {"cst": np.ascontiguousarray(cst)}


def kernel(x0, x1, W0, b0, W1, b1):
    x0 = np.ascontiguousarray(x0, dtype=np.float32)
    x1 = np.ascontiguousarray(x1, dtype=np.float32)
    W0 = np.ascontiguousarray(W0, dtype=np.float32)
    W1 = np.ascontiguousarray(W1, dtype=np.float32)
    b0 = np.asarray(b0, dtype=np.float32)
    b1 = np.asarray(b1, dtype=np.float32)
    B = x0.shape[0]
    assert B == 8, f"expected batch 8, got {B}"

    use_bias = bool(np.any(b0) or np.any(b1))
    nc = _get_program(use_bias)

    shared = _host_consts()
    for li, (W, bb) in enumerate(zip((W0, W1), (b0, b1))):
        wt = np.ascontiguousarray(W.T)  # [C, 1548]
        if use_bias:
            wt = np.concatenate([wt, bb.reshape(1, -1)], axis=0)
        shared[f"wt{li}"] = wt.astype(np.float16)

    in_maps = []
    for i in range(B):
        m = dict(shared)
        for li, (x, lv) in enumerate(zip((x0, x1), LEVELS)):
            xs = x[i].reshape(lv["C"], lv["HW"])
            if use_bias:
                xs = np.concatenate(
                    [xs, np.ones((1, lv["HW"]), np.float32)], axis=0)
            m[f"xs{li}"] = np.ascontiguousarray(xs).astype(np.float16)
        in_maps.append(m)

    res = bass_utils.run_bass_kernel_spmd(nc, in_maps, core_ids=list(range(B)))
    return np.stack([res.results[i]["out"] for i in range(B)], axis=0)

